# revision 5
# baseline (speedup 1.0000x reference)
"""Batch-parallel attention kernel for TRN2 (8 NeuronCores).

Problem: query/keys/values [16, 2048, 128] fp32 ->
         softmax(Q K^T / sqrt(128)) @ V  [16, 2048, 128] fp32.

Sharding: batch dim split across 8 cores (2 batches per core, data
parallel), no cross-core communication.

v2 design (TensorE-paced, ~57us matmul stream):
  The steady-state bottleneck pair in v1 was ScalarE exp (67us busy)
  and TensorE (69us incl. 12us of PE transposes).  v2 removes both:
  - K^T/Q^T come from xbar DMA transposes (load fp32 -> DVE bf16 cast
    -> DRAM scratch -> dma_start_transpose), chunked in quarters for
    batch 0 so the first S matmul starts ~4us in.  No PE transposes.
  - exp is split per q-block: k-tile groups {3,3,3} -> ScalarE ACT
    exp; groups {3,3,1} -> VectorE one-pass Schraudolph (tensor_scalar
    mult+add, fp32 PSUM in, int16 out bitcast to bf16:
    bf16_bits = round(s*SCALE*log2e*128 + (127*128 - 5.5))
    gives exp with ~+-3% relative error on 7/16 of the weights; the
    ones-column denominator uses the same approximated values so the
    softmax ratio cancels most of it; measured end-to-end ~7e-3 vs
    the 2e-2 gate).
  - 36 dummy matmuls at t~0 warm the PE HAM clock gate (else the
    first ~3.4us of matmuls run at 1.2 GHz instead of 2.4).
  - Engine budget per q-block (8 q-blocks/core): TensorE 7.1us
    (16 S-MM N=512 + 64 PV-MM N=132, the pacer), ScalarE 5.8us
    (3 exp + 2 O-PSUM drains), DVE 4.7us (2.5 Schraudolph + recip),
    GpSimd (normalize mul + SWDGE scratch/out stores).
  - Batch-1 staging is emitted mid-loop (qb2/qb3) so its casts queue
    on DVE/ACT behind batch-0 q-block work, not ahead of it.
  main loop per q-block of 512 q's (as v1):
    S^T tiles = K_tile @ Q^T (bf16, fp32 PSUM), 16 k-tiles grouped
    {3,3,3,3,3,1} through a 2x3-bank PSUM rotation; exp writes bf16
    SBUF; PV: out[q, 0:132] += expS^T.T @ V_aug accumulated in PSUM,
    emission lagging the exp stream by 2 groups so TensorE always has
    ready work.  V_aug carries 4 ones-columns so PV also produces the
    softmax denominator.  Softmax max-subtraction is skipped:
    energies are ~N(0,1), safely inside exp range.
PSUM budget: S^T 2x3 banks + O 2x1 banks = 8.
"""

import math
import os
import sys

import numpy as np

sys.path.insert(0, "/opt/trn_rl_repo")

import concourse.bass as bass  # noqa: E402
import concourse.mybir as mybir  # noqa: E402
import concourse.tile as tile  # noqa: E402
from concourse import bacc  # noqa: E402
from concourse.bass_utils import run_bass_kernel_spmd  # noqa: E402

B, SEQ, D = 16, 2048, 128
NCORES = 8
BPC = B // NCORES  # batches per core
P = 128  # partitions
NKT = SEQ // P  # 16 k-tiles
QB = 512  # q-block (matmul moving free dim)
NQB = SEQ // QB
NSUB = QB // P  # q-subtiles per q-block
KGROUPS = [(0, 3), (3, 3), (6, 3), (9, 3), (12, 3), (15, 1)]  # (start, len)
DVE_GROUPS = {4, 5}  # group indices whose exp runs on VectorE (Schraudolph)
SCALE = 1.0 / math.sqrt(D)
DA = D + 4  # V augmented with 4 ones-columns
F32 = mybir.dt.float32
BF16 = mybir.dt.bfloat16
I16 = mybir.dt.int16

LOG2E = 1.4426950408889634
SCHRA_A = SCALE * LOG2E * 128.0
SCHRA_B = 127.0 * 128.0 - 5.5  # centers the (1+f)/2^f interpolation error

_cached_nc = None


def _build():
    nc = bacc.Bacc("TRN2", target_bir_lowering=False, debug=False)

    q_in = nc.dram_tensor("query", [BPC, SEQ, D], F32, kind="ExternalInput").ap()
    k_in = nc.dram_tensor("keys", [BPC, SEQ, D], F32, kind="ExternalInput").ap()
    v_in = nc.dram_tensor("values", [BPC, SEQ, D], F32, kind="ExternalInput").ap()
    out = nc.dram_tensor("out", [BPC, SEQ, D], F32, kind="ExternalOutput").ap()

    with tile.TileContext(nc) as tc:
        with (
            tc.tile_pool(name="dram", bufs=1, space="DRAM") as dram_pool,
            tc.tile_pool(name="persist", bufs=1) as persist,
            tc.tile_pool(name="stage", bufs=1) as stage,
            tc.tile_pool(name="exps", bufs=5) as exps,
            tc.tile_pool(name="epilog", bufs=4) as epilog,
            tc.tile_pool(name="psum_s", bufs=2, space="PSUM") as psum_s,
            tc.tile_pool(name="psum_o", bufs=1, space="PSUM") as psum_o,
        ):
            # ACT exp table preload (one-time ~2.7us) as early as possible.
            warm = persist.tile([P, 1], F32, tag="warm")
            warm_o = persist.tile([P, 1], BF16, tag="warm_o")
            nc.vector.memset(warm, 0.0)
            nc.scalar.activation(
                warm_o, warm, mybir.ActivationFunctionType.Exp, scale=1.0
            )

            # HAM warm-up: ~36 dummy matmuls on a zeroed bf16 tile keep the
            # PE busy during the DMA prologue so the clock gate reaches
            # K=8/8 before the first real matmul (saves ~4us of half-clock).
            wmm = persist.tile([P, P], BF16, tag="wmm")
            nc.gpsimd.memset(wmm[:], 0.0)
            o_dummy = psum_o.tile([P, 2, DA], F32, tag="o_a", name="o_dummy")
            for _ in range(36):
                nc.tensor.matmul(
                    o_dummy[:, 0, 0:P], lhsT=wmm[:], rhs=wmm[:],
                    start=True, stop=True,
                )

            # ---- staging ---------------------------------------------------
            # Per batch: fp32 loads (sync ring, contiguous full-BW chunks),
            # DVE bf16 cast, SWDGE store to DRAM scratch in natural order,
            # xbar DMA-transpose into K^T/Q^T [128 d, 2048 seq] bf16.
            # Chunking by DRAM-row blocks ("(p t) d" within each block) keeps
            # every step contiguous and lets each quarter transpose as soon
            # as its store lands.  Batch 0 uses quarters (fast first S MM on
            # the scalar ring); batch 1 uses halves with transposes on the
            # sync ring, all hidden under batch-0 compute.
            QT, KT, VA = [None] * BPC, [None] * BPC, [None] * BPC

            def stage_kq(b):
                nchunks = 4 if b == 0 else 2
                rows = SEQ // nchunks
                tpc = NKT // nchunks  # k-tiles per chunk
                tr_ring = nc.scalar if b == 0 else nc.sync

                kf = stage.tile([P, NKT, D], F32, tag="kf", name=f"kf{b}")
                qf = stage.tile([P, NKT, D], F32, tag="qf", name=f"qf{b}")
                kbf = stage.tile([P, NKT, D], BF16, tag="kbf", name=f"kbf{b}")
                qbf = stage.tile([P, NKT, D], BF16, tag="qbf", name=f"qbf{b}")
                kscr = dram_pool.tile([SEQ, D], BF16, tag=f"kscr{b}")
                qscr = dram_pool.tile([SEQ, D], BF16, tag=f"qscr{b}")
                kt_t = persist.tile([P, SEQ], BF16, tag=f"kt{b}", name=f"ktT{b}")
                qt = persist.tile([P, SEQ], BF16, tag=f"qt{b}", name=f"qtT{b}")

                def ld(f, src, c):
                    nc.sync.dma_start(
                        out=f[:, c * tpc : (c + 1) * tpc, :],
                        in_=src[c * rows : (c + 1) * rows].rearrange(
                            "(p t) d -> p t d", p=P
                        ),
                    )

                def cast_store_transpose(fbf, f, scr, dst, c):
                    lo, hi = c * tpc, (c + 1) * tpc
                    nc.vector.tensor_copy(fbf[:, lo:hi, :], f[:, lo:hi, :])
                    nc.gpsimd.dma_start(
                        out=scr[c * rows : (c + 1) * rows].rearrange(
                            "(p t) d -> p (t d)", p=P
                        ),
                        in_=fbf[:, lo:hi, :].rearrange("p t d -> p (t d)"),
                    )
                    tr_ring.dma_start_transpose(
                        out=dst[:, c * rows : (c + 1) * rows],
                        in_=scr[c * rows : (c + 1) * rows, :],
                    )

                # K chunk 0 and Q chunk 0 first (the first S group needs
                # K^T tiles 0-2 and Q^T[:, 0:512]); then the rest of K
                # (needed progressively within q-block 0), then the rest of Q.
                ld(kf, k_in[b], 0)
                ld(qf, q_in[b], 0)
                for c in range(1, nchunks):
                    ld(kf, k_in[b], c)
                for c in range(1, nchunks):
                    ld(qf, q_in[b], c)

                cast_store_transpose(kbf, kf, kscr, kt_t, 0)
                cast_store_transpose(qbf, qf, qscr, qt, 0)
                for c in range(1, nchunks):
                    cast_store_transpose(kbf, kf, kscr, kt_t, c)
                for c in range(1, nchunks):
                    cast_store_transpose(qbf, qf, qscr, qt, c)
                QT[b], KT[b] = qt, kt_t

            def stage_v(b, copy_engine):
                vf = stage.tile([P, NKT, D], F32, tag="vf", name=f"vf{b}")
                nc.sync.dma_start(
                    out=vf[:, 0:8], in_=v_in[b].rearrange("(t p) d -> p t d", p=P)[:, 0:8]
                )
                nc.sync.dma_start(
                    out=vf[:, 8:NKT],
                    in_=v_in[b].rearrange("(t p) d -> p t d", p=P)[:, 8:NKT],
                )
                va = persist.tile([P, NKT, DA], BF16, tag=f"va{b}")
                nc.gpsimd.memset(va[:, :, D:DA], 1.0)
                copy_engine(va[:, 0:8, 0:D], vf[:, 0:8, :])
                copy_engine(va[:, 8:NKT, 0:D], vf[:, 8:NKT, :])
                VA[b] = va

            stage_kq(0)
            stage_v(0, nc.vector.tensor_copy)

            # ---- main loop -------------------------------------------------
            # PV emission lags the S/exp stream by PV_LAG k-groups so
            # TensorE always has ready work while exp of the current group
            # runs on ScalarE or VectorE.
            PV_LAG = 2
            o_live = {}  # (b, qb) -> o_ps pair
            pv_queue = []  # (b, qb, k0, klen, e_s, is_last_group)

            def emit_epilogue(b, qb, o_ps):
                # Normalize straight out of PSUM on DVE (no drain copy): the
                # reciprocal+mul chain itself frees the O banks ~1.6us after
                # the last PV, well before the next q-block needs them.
                rc = epilog.tile([P, NSUB], F32, tag="rc", name=f"rc{b}{qb}")
                ob = epilog.tile([P, NSUB, D], F32, tag="ob", name=f"ob{b}{qb}")
                for half in range(2):
                    nc.vector.reciprocal(
                        rc[:, 2 * half : 2 * half + 2],
                        o_ps[half][:, :, D : D + 1].rearrange("p a b -> p (a b)"),
                    )
                for sub in range(NSUB):
                    nc.vector.tensor_scalar_mul(
                        ob[:, sub, :],
                        o_ps[sub // 2][:, sub % 2, 0:D],
                        rc[:, sub : sub + 1],
                    )
                ring = nc.gpsimd if b == 0 else nc.sync
                ring.dma_start(
                    out=out[b].rearrange("(s p) d -> p s d", p=P)[
                        :, NSUB * qb : NSUB * (qb + 1), :
                    ],
                    in_=ob[:],
                )

            def emit_pv():
                b, qb, k0, klen, e_s, last = pv_queue.pop(0)
                if k0 == 0:
                    o_live[(b, qb)] = [
                        psum_o.tile([P, 2, DA], F32, tag="o_a", name=f"oa{b}{qb}"),
                        psum_o.tile([P, 2, DA], F32, tag="o_b", name=f"ob_ps{b}{qb}"),
                    ]
                o_ps = o_live[(b, qb)]
                # Two q-subtiles share one PSUM bank.  start=True clears the
                # has_written bits of the WHOLE bank, so only the bank's
                # first matmul carries it.
                for j in range(klen):
                    kt = k0 + j
                    for sub in range(NSUB):
                        nc.tensor.matmul(
                            o_ps[sub // 2][:, sub % 2, :],
                            lhsT=e_s[:, j * QB + sub * P : j * QB + (sub + 1) * P],
                            rhs=VA[b][:, kt, :],
                            start=(kt == 0 and sub % 2 == 0),
                            stop=(kt == NKT - 1 and sub % 2 == 1),
                        )
                if last:
                    emit_epilogue(b, qb, o_live.pop((b, qb)))

            for b in range(BPC):
                for qb in range(NQB):
                    # Batch-1 staging rides under batch-0 q-blocks 2/3 so
                    # its DVE casts never block q-block 0/1 exp work.
                    if b == 0 and qb == 2:
                        stage_kq(1)
                    if b == 0 and qb == 3:
                        stage_v(1, nc.gpsimd.tensor_copy)
                    for gi, (k0, klen) in enumerate(KGROUPS):
                        s_ps = psum_s.tile(
                            [P, 3 * QB], F32, tag="s", name=f"s_{b}_{qb}_{k0}"
                        )
                        for j in range(klen):
                            kt = k0 + j
                            nc.tensor.matmul(
                                s_ps[:, j * QB : (j + 1) * QB],
                                lhsT=KT[b][:, kt * P : (kt + 1) * P],
                                rhs=QT[b][:, qb * QB : (qb + 1) * QB],
                                start=True,
                                stop=True,
                            )
                        e_s = exps.tile(
                            [P, 3 * QB], BF16, tag="es", name=f"es_{b}_{qb}_{k0}"
                        )
                        if gi in DVE_GROUPS:
                            nc.vector.tensor_scalar(
                                e_s[:, : klen * QB].bitcast(I16),
                                s_ps[:, : klen * QB],
                                SCHRA_A,
                                SCHRA_B,
                                mybir.AluOpType.mult,
                                mybir.AluOpType.add,
                            )
                        else:
                            nc.scalar.activation(
                                e_s[:, : klen * QB],
                                s_ps[:, : klen * QB],
                                mybir.ActivationFunctionType.Exp,
                                scale=SCALE,
                            )
                        pv_queue.append(
                            (b, qb, k0, klen, e_s, gi == len(KGROUPS) - 1)
                        )
                        if len(pv_queue) > PV_LAG:
                            emit_pv()
            while pv_queue:
                emit_pv()

    nc.compile()
    return nc


def _get_nc():
    global _cached_nc
    if _cached_nc is None:
        _cached_nc = _build()
    return _cached_nc


def _make_in_maps(query, keys, values):
    query = np.asarray(query, dtype=np.float32)
    keys = np.asarray(keys, dtype=np.float32)
    values = np.asarray(values, dtype=np.float32)
    in_maps = []
    for c in range(NCORES):
        sl = slice(c * BPC, (c + 1) * BPC)
        in_maps.append(
            {
                "query": np.ascontiguousarray(query[sl]),
                "keys": np.ascontiguousarray(keys[sl]),
                "values": np.ascontiguousarray(values[sl]),
            }
        )
    return in_maps


def run(query, keys, values, trace=False, tmpdir=None):
    """Run on the 8 NeuronCores; returns (output, BassKernelResults)."""
    nc = _get_nc()
    in_maps = _make_in_maps(query, keys, values)
    res = run_bass_kernel_spmd(
        nc, in_maps, list(range(NCORES)), trace=trace, tmpdir=tmpdir
    )
    outp = np.concatenate(
        [np.asarray(res.results[c]["out"]) for c in range(NCORES)], axis=0
    ).astype(np.float32)
    return outp, res


def kernel(query, keys, values):
    outp, _ = run(query, keys, values, trace=False)
    return outp
